# revision 6
# baseline (speedup 1.0000x reference)
"""Trainium2 Bass kernel for nn_DAWN_74526272520648 (moe_routing).

Self-contained: builds an 8-core SPMD Bass/Tile program, shards the sequence
(one 128-token chunk per core), replicates the neuron tables, and runs via
PJRT (axon). Candidate selection (top-64-of-4096 / top-128-of-8192 nearest
positions) is done exactly on-device with per-row threshold bisection
(counts on DVE via tensor_scalar+accum, or on ACT via sign-sum);
gating thresholds via max8 extraction (qk/v) and bisection (know).
K/V are exchanged between cores with an AllGather; causal masking uses a
host-precomputed per-core additive bias so the program is core-uniform.
"""
import functools
import numpy as np

# ---- static problem config (hardcoded per contest rules) ----
S, D, P = 1024, 1024, 32
CHUNK = 128
NCORES = 8
NQK, NV, NK = 4096, 4096, 8192
NH, DH = 16, 64
MAXK_QK, MAXK_V, MAXK_KNOW = 32, 32, 64
NCAND_QK, NCAND_V, NCAND_KNOW = 64, 64, 128

# bisection iteration counts (bracket width / f32 gap margins checked on data)
T_D = 21    # qk/v distance threshold
T_DK = 23   # know distance threshold
T_AK = 18   # know act (gate) threshold
NEG = -1e30


def _build_program(n_cores=NCORES):
    import contextlib
    import concourse.bacc as bacc
    import concourse.tile as tile
    import concourse.mybir as mybir
    from concourse.masks import make_identity

    F32 = mybir.dt.float32
    Alu = mybir.AluOpType
    Act = mybir.ActivationFunctionType

    nc = bacc.Bacc('TRN2', target_bir_lowering=False, debug=False,
                   num_devices=n_cores)

    # ---------------- DRAM I/O ----------------
    def inp(name, shape):
        return nc.dram_tensor(name, shape, F32, kind='ExternalInput')

    x_c = inp('x_c', [CHUNK, D])
    xT_c = inp('xT_c', [D, CHUNK])
    posT_qk = inp('posT_qk', [P + 1, CHUNK])
    posT_v = inp('posT_v', [P + 1, CHUNK])
    posT_know = inp('posT_know', [P + 1, CHUNK])
    ntq = inp('negtau_q', [CHUNK, 1])
    ntk = inp('negtau_k', [CHUNK, 1])
    ntv = inp('negtau_v', [CHUNK, 1])
    ntkn = inp('negtau_know', [CHUNK, 1])
    cbias = inp('causal_bias', [CHUNK, S])
    nposT_qk = inp('nposT_qk', [P + 1, NQK])
    nposT_v = inp('nposT_v', [P + 1, NV])
    nposT_know = inp('nposT_know', [P + 1, NK])
    tabT_qk = inp('tabT_qk', [D, NQK])
    tab_qk = inp('tab_qk', [NQK, D])
    tabT_v = inp('tabT_v', [D, NV])
    tab_v = inp('tab_v', [NV, D])
    tabT_know = inp('tabT_know', [D, NK])
    tab_know = inp('tab_know', [NK, D])
    expO = inp('expO', [D, D])

    out_c = nc.dram_tensor('out_c', [CHUNK, D], F32, kind='ExternalOutput')
    loss_c = nc.dram_tensor('loss_c', [CHUNK, 8], F32, kind='ExternalOutput')

    kt_bounce = nc.dram_tensor('kt_bounce', [D, CHUNK], F32, kind='Internal')
    v_bounce = nc.dram_tensor('v_bounce', [CHUNK, D], F32, kind='Internal')
    kt_all = nc.dram_tensor('kt_all', [n_cores, D, CHUNK], F32, kind='Internal',
                            addr_space='Shared')
    v_all = nc.dram_tensor('v_all', [n_cores, CHUNK, D], F32, kind='Internal',
                           addr_space='Shared')
    groups = [list(range(n_cores))]

    with tile.TileContext(nc) as tc:
        with contextlib.ExitStack() as ctx:
            big = ctx.enter_context(tc.tile_pool(name='big', bufs=8))
            sm = ctx.enter_context(tc.tile_pool(name='sm', bufs=24))
            st_t = ctx.enter_context(tc.tile_pool(name='st_t', bufs=4))
            st_n = ctx.enter_context(tc.tile_pool(name='st_n', bufs=3))
            persist = ctx.enter_context(tc.tile_pool(name='persist', bufs=1))
            ps = ctx.enter_context(tc.tile_pool(name='ps', bufs=8, space='PSUM'))

            def big_tile():
                return big.tile([CHUNK, 4096], F32, tag='big', name='bigt')

            def sm_tile(w=1):
                return sm.tile([CHUNK, 16], F32, tag='sm', name='smt')[:, 0:w]

            def ps_tile():
                return ps.tile([128, 512], F32, tag='ps', name='pst')

            # ---------- persistent small loads ----------
            ident = persist.tile([128, 128], F32, tag='ident')
            make_identity(nc, ident[:])
            xT = persist.tile([128, 8 * CHUNK], F32, tag='xT')
            for dt in range(8):
                nc.sync.dma_start(xT[:, dt*CHUNK:(dt+1)*CHUNK],
                                  xT_c[dt*128:(dt+1)*128, :])
            xn = persist.tile([CHUNK, D], F32, tag='xn')
            nc.sync.dma_start(xn[:], x_c[:])
            cb = persist.tile([CHUNK, S], F32, tag='cb')
            nc.sync.dma_start(cb[:], cbias[:])
            taus = {}
            for nm, t_ in (('q', ntq), ('k', ntk), ('v', ntv), ('kn', ntkn)):
                tt = persist.tile([CHUNK, 1], F32, tag='tau' + nm)
                nc.sync.dma_start(tt[:], t_[:])
                taus[nm] = tt
            pos_t = {}
            for nm, t_ in (('qk', posT_qk), ('v', posT_v), ('kn', posT_know)):
                tt = persist.tile([P + 1, CHUNK], F32, tag='pos' + nm)
                nc.sync.dma_start(tt[:], t_[:])
                pos_t[nm] = tt
            loss_sb = persist.tile([CHUNK, 8], F32, tag='loss')
            nc.vector.memset(loss_sb[:], 0.0)

            # ---------- helpers ----------
            def dve_ts(out, in0, s1, s2, op0, op1=None, accum=None):
                kw = {}
                if op1 is not None:
                    kw['op1'] = op1
                if accum is not None:
                    kw['accum_out'] = accum
                nc.vector.tensor_scalar(out, in0, s1, s2, op0=op0, **kw)

            def pool_ts(out, in0, s1, s2, op0, op1=None):
                kw = {'op1': op1} if op1 is not None else {}
                nc.gpsimd.tensor_scalar(out, in0, s1, s2, op0=op0, **kw)

            def pool_tt(out, a, b, op):
                nc.gpsimd.tensor_tensor(out, a, b, op=op)

            def evict(dst, src):  # PSUM -> SBUF on ACT (Identity works, Copy broken)
                nc.scalar.activation(dst, src, Act.Identity, bias=0.0, scale=1.0)

            def count_le_dve(halves, mid, hw):
                tot = None
                for h_ in halves:
                    cnt = sm_tile()
                    junk = big_tile()
                    dve_ts(junk[:, :hw], h_, mid[:, 0:1], 0.0, Alu.is_le,
                           Alu.add, cnt[:, 0:1])
                    if tot is None:
                        tot = cnt
                    else:
                        c2 = sm_tile()
                        pool_tt(c2[:, 0:1], tot[:, 0:1], cnt[:, 0:1], Alu.add)
                        tot = c2
                return tot

            def count_sgn_act(halves, mid, hw):
                """sum(sign(a - mid)) over the row (all halves)."""
                nmid = sm_tile()
                pool_ts(nmid[:, 0:1], mid[:, 0:1], -1.0, None, Alu.mult)
                tot = None
                for h_ in halves:
                    sg = sm_tile()
                    junk = big_tile()
                    nc.scalar.activation(junk[:, :hw], h_, Act.Sign,
                                         bias=nmid[:, 0:1], scale=1.0,
                                         accum_out=sg[:, 0:1])
                    if tot is None:
                        tot = sg
                    else:
                        t2 = sm_tile()
                        pool_tt(t2[:, 0:1], tot[:, 0:1], sg[:, 0:1], Alu.add)
                        tot = t2
                return tot

            def bisect_generic(Ainit, Binit, iters, make_pred):
                """pred true -> A = mid ; else B = mid. returns final A."""
                A, B = Ainit, Binit
                for _ in range(iters):
                    mid0 = sm_tile()
                    pool_tt(mid0[:, 0:1], A[:, 0:1], B[:, 0:1], Alu.add)
                    mid = sm_tile()
                    pool_ts(mid[:, 0:1], mid0[:, 0:1], 0.5, None, Alu.mult)
                    pred = make_pred(mid)
                    predn = sm_tile()
                    pool_ts(predn[:, 0:1], pred[:, 0:1], -1.0, 1.0, Alu.mult, Alu.add)
                    dA = sm_tile(); dB = sm_tile()
                    pool_tt(dA[:, 0:1], mid[:, 0:1], A[:, 0:1], Alu.subtract)
                    pool_tt(dB[:, 0:1], mid[:, 0:1], B[:, 0:1], Alu.subtract)
                    uA = sm_tile(); uB = sm_tile()
                    pool_tt(uA[:, 0:1], pred[:, 0:1], dA[:, 0:1], Alu.mult)
                    pool_tt(uB[:, 0:1], predn[:, 0:1], dB[:, 0:1], Alu.mult)
                    A2 = sm_tile(); B2 = sm_tile()
                    pool_tt(A2[:, 0:1], A[:, 0:1], uA[:, 0:1], Alu.add)
                    pool_tt(B2[:, 0:1], B[:, 0:1], uB[:, 0:1], Alu.add)
                    A, B = A2, B2
                return A

            def seg_reduce(halves, hw, nseg_per_half, op):
                tot = len(halves) * nseg_per_half
                segs = sm.tile([CHUNK, 128], F32, tag='segs', bufs=2, name='segs')
                for i, h_ in enumerate(halves):
                    hv = h_.rearrange("p (s i) -> p s i", s=nseg_per_half)
                    nc.vector.tensor_reduce(
                        segs[:, i*nseg_per_half:(i+1)*nseg_per_half], hv, op=op,
                        axis=mybir.AxisListType.X)
                return segs, tot

            def row_reduce(t_, op, w):
                r = sm_tile()
                nc.vector.tensor_reduce(r[:, 0:1], t_[:, :w], op=op,
                                        axis=mybir.AxisListType.X)
                return r

            # ================= one neuron-table stage =================
            def table_stage(nn, nhalves, npos_d, pos_tile, tabT_d, tab_d,
                            lhsT_tile, k_cand, t_dist_iters, gates,
                            dist_on_act=False):
                hw = nn // nhalves
                nt_h = hw // 512
                # --- distances ---
                dS = []
                for hf in range(nhalves):
                    dst = big_tile()
                    for ntile in range(nt_h):
                        pd = ps_tile()
                        col0 = hf * hw + ntile * 512
                        np_ = st_t.tile([P + 1, 512], F32, tag='st_t', name='np_tile')
                        nc.sync.dma_start(np_[:], npos_d[:, col0:col0+512])
                        nc.tensor.matmul(pd[:], pos_tile[:], np_[:],
                                         start=True, stop=True)
                        evict(dst[:, ntile*512:(ntile+1)*512], pd[:])
                    dS.append(dst)
                dhalves = [t_[:, :hw] for t_ in dS]
                # --- candidate threshold (bisect; bracket from segment minima) ---
                segs, totseg = seg_reduce(dhalves, hw, hw // 64, Alu.min)
                lo0 = row_reduce(segs, Alu.min, totseg)
                hi0 = row_reduce(segs, Alu.max, totseg)

                def dist_pred(mid):
                    pred = sm_tile()
                    if dist_on_act:
                        sg = count_sgn_act(dhalves, mid, hw)
                        pool_ts(pred[:, 0:1], sg[:, 0:1],
                                float(nn - 2*k_cand + 1), None, Alu.is_le)
                    else:
                        cnt = count_le_dve(dhalves, mid, hw)
                        pool_ts(pred[:, 0:1], cnt[:, 0:1],
                                float(k_cand) - 0.5, None, Alu.is_ge)
                    return pred

                t64 = bisect_generic(hi0, lo0, t_dist_iters, dist_pred)

                # --- act matmuls + masked eviction ---
                am = []
                for hf in range(nhalves):
                    mn = big_tile()
                    dve_ts(mn[:, :hw], dS[hf][:, :hw], t64[:, 0:1], NEG,
                           Alu.is_gt, Alu.mult)
                    dst = big_tile()
                    pa = [ps_tile() for _ in range(nt_h)]
                    for dt in range(8):
                        for ntile in range(nt_h):
                            col0 = hf * hw + ntile * 512
                            tt_ = st_t.tile([128, 512], F32, tag='st_t', name='stt_tile')
                            nc.sync.dma_start(tt_[:],
                                              tabT_d[dt*128:(dt+1)*128, col0:col0+512])
                            nc.tensor.matmul(
                                pa[ntile][:],
                                lhsT_tile[:, dt*CHUNK:(dt+1)*CHUNK], tt_[:],
                                start=(dt == 0), stop=(dt == 7))
                    for ntile in range(nt_h):
                        nc.vector.scalar_tensor_tensor(
                            dst[:, ntile*512:(ntile+1)*512], in0=pa[ntile][:],
                            scalar=0.0, in1=mn[:, ntile*512:(ntile+1)*512],
                            op0=Alu.add, op1=Alu.add)
                    am.append(dst)
                ahalves = [t_[:, :hw] for t_ in am]

                # --- gate threshold a_thr + row max act (shared by all gates:
                #     the keep-set is the top-k_keep by act, tau-independent) ---
                k_keep = gates[0][2]
                if nhalves == 1 and k_keep <= 32:
                    rounds = k_keep // 8
                    m8s = []
                    prev = am[0][:, :hw]
                    for r in range(rounds):
                        m8 = sm.tile([CHUNK, 8], F32, tag='m8', bufs=8, name='m8')
                        nc.vector.max(out=m8[:], in_=prev)
                        m8s.append(m8)
                        if r < rounds - 1:
                            wk = big_tile()
                            nc.vector.match_replace(
                                out=wk[:, :hw], in_to_replace=m8[:],
                                in_values=prev, imm_value=NEG)
                            prev = wk[:, :hw]
                    a_thr = m8s[-1][:, 7:8]
                    a_max = m8s[0][:, 0:1]
                else:
                    sga, totsg = seg_reduce(ahalves, hw, hw // 128, Alu.max)
                    alo = row_reduce(sga, Alu.min, totsg)
                    ahi = row_reduce(sga, Alu.max, totsg)
                    a_max = ahi

                    def act_pred(mid):
                        sg = count_sgn_act(ahalves, mid, hw)
                        pred = sm_tile()
                        pool_ts(pred[:, 0:1], sg[:, 0:1],
                                float(2*k_keep - 1 - nn), None, Alu.is_ge)
                        return pred
                    a_thr = bisect_generic(alo, ahi, T_AK, act_pred)

                wTs = {}
                for gname, negtau, k_keep, want_loss, loss_base in gates:
                    # --- alpha = tanh(max_eg) / (sum(eg_kept) + 1e-8) ---
                    t1 = sm_tile()
                    nc.scalar.activation(t1[:, 0:1], a_max[:, 0:1], Act.Exp,
                                         bias=negtau[:, 0:1], scale=1.0)
                    meg = sm_tile()
                    dve_ts(meg[:, 0:1], t1[:, 0:1], 1.0, 0.0, Alu.subtract, Alu.max)
                    stren = sm_tile()
                    nc.scalar.activation(stren[:, 0:1], meg[:, 0:1], Act.Tanh,
                                         bias=0.0, scale=1.0)
                    Ssum = None
                    egks = []
                    for hf in range(nhalves):
                        e = big_tile()
                        nc.scalar.activation(e[:, :hw], am[hf][:, :hw], Act.Exp,
                                             bias=negtau[:, 0:1], scale=1.0)
                        eg = big_tile()
                        dve_ts(eg[:, :hw], e[:, :hw], 1.0, 0.0, Alu.subtract, Alu.max)
                        egk = big_tile()
                        nc.vector.scalar_tensor_tensor(
                            egk[:, :hw], in0=am[hf][:, :hw], scalar=a_thr[:, 0:1],
                            in1=eg[:, :hw], op0=Alu.is_ge, op1=Alu.mult)
                        s_ = sm_tile()
                        nc.vector.tensor_reduce(s_[:, 0:1], egk[:, :hw], op=Alu.add,
                                                axis=mybir.AxisListType.X)
                        if Ssum is None:
                            Ssum = s_
                        else:
                            s2 = sm_tile()
                            pool_tt(s2[:, 0:1], Ssum[:, 0:1], s_[:, 0:1], Alu.add)
                            Ssum = s2
                        egks.append(egk)
                    sp = sm_tile()
                    pool_ts(sp[:, 0:1], Ssum[:, 0:1], 1e-8, None, Alu.add)
                    rec = sm_tile()
                    nc.vector.reciprocal(rec[:, 0:1], sp[:, 0:1])
                    alpha = sm_tile()
                    pool_tt(alpha[:, 0:1], stren[:, 0:1], rec[:, 0:1], Alu.mult)

                    if want_loss:
                        ldt = None
                        for hf in range(nhalves):
                            pj = big_tile()
                            nc.vector.tensor_tensor(pj[:, :hw], egks[hf][:, :hw],
                                                    dS[hf][:, :hw], op=Alu.mult)
                            l_ = sm_tile()
                            nc.vector.tensor_reduce(l_[:, 0:1], pj[:, :hw],
                                                    op=Alu.add,
                                                    axis=mybir.AxisListType.X)
                            if ldt is None:
                                ldt = l_
                            else:
                                l2 = sm_tile()
                                pool_tt(l2[:, 0:1], ldt[:, 0:1], l_[:, 0:1], Alu.add)
                                ldt = l2
                        pool_tt(loss_sb[:, loss_base:loss_base+1],
                                ldt[:, 0:1], alpha[:, 0:1], Alu.mult)
                        pool_tt(loss_sb[:, loss_base+1:loss_base+2],
                                alpha[:, 0:1], Ssum[:, 0:1], Alu.mult)

                    # --- w = act*alpha*eg_kept, transposed per half ---
                    wT_halves = []
                    for hf in range(nhalves):
                        w_ = big_tile()
                        nc.vector.scalar_tensor_tensor(
                            w_[:, :hw], in0=am[hf][:, :hw], scalar=alpha[:, 0:1],
                            in1=egks[hf][:, :hw], op0=Alu.mult, op1=Alu.mult)
                        wT = big_tile()
                        nblk = hw // 128
                        for b0 in range(0, nblk, 4):
                            pt = ps_tile()
                            for b in range(b0, b0 + 4):
                                nc.tensor.transpose(pt[:, (b-b0)*128:(b-b0+1)*128],
                                                    w_[:, b*128:(b+1)*128], ident[:])
                            evict(wT[:, b0*128:(b0+4)*128], pt[:])
                        wT_halves.append(wT)
                    wTs[gname] = wT_halves

                # --- recon: stream native table once; matmul all gates ---
                outs = {g: [ps_tile(), ps_tile()] for g in wTs}
                ntt = nn // 128
                blocks_per_half = hw // 128
                for ntile in range(ntt):
                    tnat = st_n.tile([128, D], F32, tag='st_n', name='stn_tile')
                    nc.sync.dma_start(tnat[:], tab_d[ntile*128:(ntile+1)*128, :])
                    hf = ntile // blocks_per_half
                    lb = ntile % blocks_per_half
                    for g in wTs:
                        for half in range(2):
                            nc.tensor.matmul(
                                outs[g][half][:],
                                wTs[g][hf][:, lb*128:(lb+1)*128],
                                tnat[:, half*512:(half+1)*512],
                                start=(ntile == 0), stop=(ntile == ntt - 1))
                res = {}
                for g in wTs:
                    dst = big_tile()
                    evict(dst[:, 0:512], outs[g][0][:])
                    evict(dst[:, 512:1024], outs[g][1][:])
                    res[g] = dst
                return res

            # ---------------- qk stage ----------------
            qk_out = table_stage(
                NQK, 1, nposT_qk, pos_t['qk'], tabT_qk, tab_qk,
                xT, NCAND_QK, T_D,
                gates=[('Q', taus['q'], MAXK_QK, True, 0),
                       ('K', taus['k'], MAXK_QK, False, 0)])

            # ---------------- v stage (distance counts on ACT) ----------------
            v_out = table_stage(
                NV, 1, nposT_v, pos_t['v'], tabT_v, tab_v,
                xT, NCAND_V, T_D,
                gates=[('V', taus['v'], MAXK_V, True, 2)],
                dist_on_act=True)

            # ---------------- transpose Q,K ; allgather K,V ----------------
            def transpose_1024(src, dst):
                for b0 in range(0, 8, 4):
                    pt = ps_tile()
                    for b in range(b0, b0+4):
                        nc.tensor.transpose(pt[:, (b-b0)*128:(b-b0+1)*128],
                                            src[:, b*128:(b+1)*128], ident[:])
                    evict(dst[:, b0*128:(b0+4)*128], pt[:])

            QT = persist.tile([128, 8 * CHUNK], F32, tag='QT')
            transpose_1024(qk_out['Q'], QT)
            KT = big_tile()
            transpose_1024(qk_out['K'], KT)
            for dt in range(8):
                nc.sync.dma_start(kt_bounce[dt*128:(dt+1)*128, :],
                                  KT[:, dt*128:(dt+1)*128])
            nc.sync.dma_start(v_bounce[:], v_out['V'][:, 0:1024])
            nc.gpsimd.collective_compute(
                'AllGather', mybir.AluOpType.bypass, replica_groups=groups,
                ins=[kt_bounce[:]], outs=[kt_all[:]])
            nc.gpsimd.collective_compute(
                'AllGather', mybir.AluOpType.bypass, replica_groups=groups,
                ins=[v_bounce[:]], outs=[v_all[:]])

            # ---------------- attention ----------------
            oT = persist.tile([128, 8 * CHUNK], F32, tag='oT')
            for g in range(4):
                KTg = big_tile()
                Vg = big_tile()
                for j in range(n_cores):
                    for b in range(2):
                        nc.sync.dma_start(
                            KTg[:, (j*2+b)*128:(j*2+b+1)*128],
                            kt_all[j, 128*(2*g+b):128*(2*g+b+1), :])
                    nc.sync.dma_start(Vg[:, j*256:(j+1)*256],
                                      v_all[j, :, 256*g:256*(g+1)])
                for hh in range(4):
                    h = 4*g + hh
                    pb = (h % 2) * 64
                    b = (h % 4) // 2
                    sc0, sc1 = ps_tile(), ps_tile()
                    for j in range(8):
                        dstp = sc0 if j < 4 else sc1
                        nc.tensor.matmul(
                            dstp[:, (j % 4)*128:(j % 4 + 1)*128],
                            QT[pb:pb+64, 128*(h//2):128*(h//2)+128],
                            KTg[pb:pb+64, (j*2+b)*128:(j*2+b+1)*128],
                            start=True, stop=True)
                    ssb = big_tile()
                    nc.vector.scalar_tensor_tensor(
                        ssb[:, 0:512], in0=sc0[:], scalar=0.0,
                        in1=cb[:, 0:512], op0=Alu.add, op1=Alu.add)
                    nc.vector.scalar_tensor_tensor(
                        ssb[:, 512:1024], in0=sc1[:], scalar=0.0,
                        in1=cb[:, 512:1024], op0=Alu.add, op1=Alu.add)
                    rm = row_reduce(ssb, Alu.max, 1024)
                    nm = sm_tile()
                    pool_ts(nm[:, 0:1], rm[:, 0:1], -0.125, None, Alu.mult)
                    e = big_tile()
                    ssum = sm_tile()
                    nc.scalar.activation(e[:, 0:1024], ssb[:, 0:1024], Act.Exp,
                                         bias=nm[:, 0:1], scale=0.125,
                                         accum_out=ssum[:, 0:1])
                    winv = sm_tile()
                    nc.vector.reciprocal(winv[:, 0:1], ssum[:, 0:1])
                    wn = big_tile()
                    dve_ts(wn[:, 0:1024], e[:, 0:1024], winv[:, 0:1], None, Alu.mult)
                    wT_ = big_tile()
                    for b0 in range(0, 8, 4):
                        pt = ps_tile()
                        for j in range(b0, b0+4):
                            nc.tensor.transpose(pt[:, (j-b0)*128:(j-b0+1)*128],
                                                wn[:, j*128:(j+1)*128], ident[:])
                        evict(wT_[:, b0*128:(b0+4)*128], pt[:])
                    po = ps.tile([64, 128], F32, tag='ps', name='po')
                    for j in range(8):
                        nc.tensor.matmul(po[:],
                                         Vg[:, j*256 + hh*64: j*256 + hh*64 + 64],
                                         wT_[:, j*128:(j+1)*128],
                                         start=(j == 0), stop=(j == 7))
                    evict(oT[pb:pb+64, 128*(h//2):128*(h//2)+128], po[:])

            # attn_out = oT.T @ expO ; h = x + attn_out
            pao = [ps_tile(), ps_tile()]
            for dit in range(8):
                eo = st_n.tile([128, D], F32, tag='st_n', name='stn_tile')
                nc.sync.dma_start(eo[:], expO[dit*128:(dit+1)*128, :])
                for half in range(2):
                    nc.tensor.matmul(pao[half][:], oT[:, dit*128:(dit+1)*128],
                                     eo[:, half*512:(half+1)*512],
                                     start=(dit == 0), stop=(dit == 7))
            h_sb = persist.tile([CHUNK, D], F32, tag='h')
            for half in range(2):
                nc.vector.scalar_tensor_tensor(
                    h_sb[:, half*512:(half+1)*512], in0=pao[half][:], scalar=0.0,
                    in1=xn[:, half*512:(half+1)*512], op0=Alu.add, op1=Alu.add)
            hT = persist.tile([128, 8 * CHUNK], F32, tag='hT')
            transpose_1024(h_sb, hT)

            # ---------------- know stage ----------------
            kn_out = table_stage(
                NK, 2, nposT_know, pos_t['kn'], tabT_know, tab_know,
                hT, NCAND_KNOW, T_DK,
                gates=[('KN', taus['kn'], MAXK_KNOW, True, 4)])

            fin = big_tile()
            nc.vector.tensor_tensor(fin[:, 0:1024], kn_out['KN'][:, 0:1024],
                                    h_sb[:, 0:1024], op=Alu.add)
            nc.sync.dma_start(out_c[:], fin[:, 0:1024])
            nc.sync.dma_start(loss_c[:], loss_sb[:])

    nc.finalize()
    return nc


@functools.lru_cache(maxsize=1)
def _get_nc(n_cores=NCORES):
    return _build_program(n_cores)


def _host_prep(inputs):
    x = np.ascontiguousarray(inputs['x'][0], np.float32)
    xT = np.ascontiguousarray(x.T)

    def aug_posT(pos):
        a = np.empty((P + 1, S), np.float32)
        a[:P] = pos.T
        a[P] = 1.0
        return a

    def aug_nposT(npos):
        n = npos.shape[0]
        a = np.empty((P + 1, n), np.float32)
        a[:P] = -2.0 * npos.T
        a[P] = (npos.astype(np.float32)**2).sum(-1)
        return a

    pq = aug_posT(inputs['qk_pos'][0])
    pv = aug_posT(inputs['v_pos'][0])
    pk = aug_posT(inputs['know_pos'][0])
    gpos = np.arange(S, dtype=np.int64)

    rep = {
        'nposT_qk': aug_nposT(inputs['npos_qk']),
        'nposT_v': aug_nposT(inputs['npos_v']),
        'nposT_know': aug_nposT(inputs['npos_know']),
        'tabT_qk': np.ascontiguousarray(inputs['qk_neurons'].T),
        'tab_qk': np.ascontiguousarray(inputs['qk_neurons']),
        'tabT_v': np.ascontiguousarray(inputs['v_neurons'].T),
        'tab_v': np.ascontiguousarray(inputs['v_neurons']),
        'tabT_know': np.ascontiguousarray(inputs['know_neurons'].T),
        'tab_know': np.ascontiguousarray(inputs['know_neurons']),
        'expO': np.ascontiguousarray(inputs['expand_O']),
    }
    per_core = []
    for c in range(NCORES):
        sl = slice(c*CHUNK, (c+1)*CHUNK)
        cbm = np.where(gpos[None, :] <= gpos[sl][:, None], 0.0, NEG).astype(np.float32)
        m = {
            'x_c': x[sl].copy(),
            'xT_c': np.ascontiguousarray(xT[:, sl]),
            'posT_qk': np.ascontiguousarray(pq[:, sl]),
            'posT_v': np.ascontiguousarray(pv[:, sl]),
            'posT_know': np.ascontiguousarray(pk[:, sl]),
            'negtau_q': np.ascontiguousarray(-inputs['tau_q'][0][sl]),
            'negtau_k': np.ascontiguousarray(-inputs['tau_k'][0][sl]),
            'negtau_v': np.ascontiguousarray(-inputs['tau_v'][0][sl]),
            'negtau_know': np.ascontiguousarray(-inputs['tau_know'][0][sl]),
            'causal_bias': cbm,
        }
        m.update(rep)
        per_core.append(m)
    return per_core


_PER_CORE = {'x_c', 'xT_c', 'posT_qk', 'posT_v', 'posT_know', 'negtau_q',
             'negtau_k', 'negtau_v', 'negtau_know', 'causal_bias'}


@functools.lru_cache(maxsize=1)
def _make_runner():
    """jit-compiled SPMD executable with replicated table inputs.

    Returns (fn, in_names, out_names, zero_outs). fn takes device (or host)
    arrays in in_names order followed by zero output buffers, returns the
    gathered outputs (concat over cores on axis 0).
    """
    import jax
    from jax.sharding import Mesh, PartitionSpec
    from jax.experimental.shard_map import shard_map
    from concourse import bass2jax
    import concourse.mybir as mybir

    nc = _get_nc()
    bass2jax.install_neuronx_cc_hook()

    pname = nc.partition_id_tensor.name if nc.partition_id_tensor else None
    in_names, out_names, out_avals, zero_outs = [], [], [], []
    for alloc in nc.m.functions[0].allocations:
        if not isinstance(alloc, mybir.MemoryLocationSet):
            continue
        name = alloc.memorylocations[0].name
        if alloc.kind == 'ExternalInput':
            if name != pname:
                in_names.append(name)
        elif alloc.kind == 'ExternalOutput':
            out_names.append(name)
            shape = tuple(alloc.tensor_shape)
            dt = mybir.dt.np(alloc.dtype)
            out_avals.append(jax.core.ShapedArray(shape, dt))
            zero_outs.append(np.zeros(shape, dt))
    n_params = len(in_names)
    all_in = in_names + out_names + ([pname] if pname else [])

    def _body(*args):
        operands = list(args)
        if pname:
            operands.append(bass2jax.partition_id_tensor())
        outs = bass2jax._bass_exec_p.bind(
            *operands, out_avals=tuple(out_avals), in_names=tuple(all_in),
            out_names=tuple(out_names), lowering_input_output_aliases=(),
            sim_require_finite=True, sim_require_nnan=True, nc=nc)
        return tuple(outs)

    mesh = Mesh(np.asarray(jax.devices()[:NCORES]), ('core',))
    in_specs = tuple(
        PartitionSpec('core') if nm in _PER_CORE else PartitionSpec()
        for nm in in_names) + (PartitionSpec('core'),) * len(out_names)
    out_specs = (PartitionSpec('core'),) * len(out_names)
    fn = jax.jit(
        shard_map(_body, mesh=mesh, in_specs=in_specs, out_specs=out_specs,
                  check_rep=False),
        donate_argnums=tuple(range(n_params, n_params + len(out_names))))
    return fn, in_names, out_names, zero_outs


def device_args(inputs):
    """Host arrays in runner order: per-core inputs concat over cores."""
    per_core = _host_prep(inputs)
    _, in_names, _, zero_outs = _make_runner()
    args = []
    for nm in in_names:
        if nm in _PER_CORE:
            args.append(np.concatenate([per_core[c][nm] for c in range(NCORES)],
                                       axis=0))
        else:
            args.append(per_core[0][nm])
    return args, [z.copy() for z in zero_outs]


def run_device(inputs):
    fn, in_names, out_names, zero_outs = _make_runner()
    args, zo = device_args(inputs)
    res = fn(*args, *zo)
    res = {nm: np.asarray(r) for nm, r in zip(out_names, res)}
    outs = [res['out_c'][c*CHUNK:(c+1)*CHUNK] for c in range(NCORES)]
    losses = [res['loss_c'][c*CHUNK:(c+1)*CHUNK] for c in range(NCORES)]
    return outs, losses


def assemble(inputs, outs, losses):
    out = np.concatenate(outs, axis=0)[None]
    lp = np.concatenate(losses, axis=0)
    p2_qk = (inputs['qk_pos'][0].astype(np.float32)**2).sum(-1)
    p2_v = (inputs['v_pos'][0].astype(np.float32)**2).sum(-1)
    p2_k = (inputs['know_pos'][0].astype(np.float32)**2).sum(-1)
    attn_loss = (lp[:, 0] + p2_qk*lp[:, 1] + lp[:, 2] + p2_v*lp[:, 3]).sum() / (8*128*64)
    know_loss = (lp[:, 4] + p2_k*lp[:, 5]).sum() / (8*128*128)
    return out.astype(np.float32), np.float32(attn_loss + know_loss)


def kernel(**inputs):
    inputs = {k: np.asarray(v, np.float32) for k, v in inputs.items()}
    outs, losses = run_device(inputs)
    return assemble(inputs, outs, losses)


# revision 7
# speedup vs baseline: 2.7431x; 2.7431x over previous
"""Trainium2 Bass kernel for nn_DAWN_74526272520648 (moe_routing).

Self-contained: builds an 8-core SPMD Bass/Tile program, shards the sequence
(one 128-token chunk per core), replicates the neuron tables, and runs via
PJRT (axon). Candidate selection (top-64-of-4096 / top-128-of-8192 nearest
positions) is done exactly on-device with per-row threshold bisection
(counts on DVE via tensor_scalar+accum, or on ACT via sign-sum);
gating thresholds via max8 extraction (qk/v) and bisection (know).
K/V are exchanged between cores with an AllGather; causal masking uses a
host-precomputed per-core additive bias so the program is core-uniform.
"""
import functools
import numpy as np

# ---- static problem config (hardcoded per contest rules) ----
S, D, P = 1024, 1024, 32
CHUNK = 128
NCORES = 8
NQK, NV, NK = 4096, 4096, 8192
NH, DH = 16, 64
MAXK_QK, MAXK_V, MAXK_KNOW = 32, 32, 64
NCAND_QK, NCAND_V, NCAND_KNOW = 64, 64, 128

# bisection iteration counts (bracket width / f32 gap margins checked on data)
T_D = 21    # qk/v distance threshold
T_DK = 23   # know distance threshold
T_AK = 18   # know act (gate) threshold
NEG = -1e30


def _build_program(n_cores=NCORES):
    import contextlib
    import concourse.bacc as bacc
    import concourse.tile as tile
    import concourse.mybir as mybir
    from concourse.masks import make_identity

    F32 = mybir.dt.float32
    Alu = mybir.AluOpType
    Act = mybir.ActivationFunctionType

    nc = bacc.Bacc('TRN2', target_bir_lowering=False, debug=False,
                   num_devices=n_cores)

    # ---------------- DRAM I/O ----------------
    def inp(name, shape):
        return nc.dram_tensor(name, shape, F32, kind='ExternalInput')

    x_c = inp('x_c', [CHUNK, D])
    xT_c = inp('xT_c', [D, CHUNK])
    posT_qk = inp('posT_qk', [P + 1, CHUNK])
    posT_v = inp('posT_v', [P + 1, CHUNK])
    posT_know = inp('posT_know', [P + 1, CHUNK])
    ntq = inp('negtau_q', [CHUNK, 1])
    ntk = inp('negtau_k', [CHUNK, 1])
    ntv = inp('negtau_v', [CHUNK, 1])
    ntkn = inp('negtau_know', [CHUNK, 1])
    cbias = inp('causal_bias', [CHUNK, S])
    nposT_qk = inp('nposT_qk', [P + 1, NQK])
    nposT_v = inp('nposT_v', [P + 1, NV])
    nposT_know = inp('nposT_know', [P + 1, NK])
    tabT_qk = inp('tabT_qk', [D, NQK])
    tab_qk = inp('tab_qk', [NQK, D])
    tabT_v = inp('tabT_v', [D, NV])
    tab_v = inp('tab_v', [NV, D])
    tabT_know = inp('tabT_know', [D, NK])
    tab_know = inp('tab_know', [NK, D])
    expO = inp('expO', [D, D])

    out_c = nc.dram_tensor('out_c', [CHUNK, D], F32, kind='ExternalOutput')
    loss_c = nc.dram_tensor('loss_c', [CHUNK, 8], F32, kind='ExternalOutput')

    kt_bounce = nc.dram_tensor('kt_bounce', [D, CHUNK], F32, kind='Internal')
    v_bounce = nc.dram_tensor('v_bounce', [CHUNK, D], F32, kind='Internal')
    kt_all = nc.dram_tensor('kt_all', [n_cores, D, CHUNK], F32, kind='Internal',
                            addr_space='Shared')
    v_all = nc.dram_tensor('v_all', [n_cores, CHUNK, D], F32, kind='Internal',
                           addr_space='Shared')
    groups = [list(range(n_cores))]

    with tile.TileContext(nc) as tc:
        with contextlib.ExitStack() as ctx:
            big = ctx.enter_context(tc.tile_pool(name='big', bufs=8))
            sm = ctx.enter_context(tc.tile_pool(name='sm', bufs=24))
            st_t = ctx.enter_context(tc.tile_pool(name='st_t', bufs=4))
            st_n = ctx.enter_context(tc.tile_pool(name='st_n', bufs=3))
            persist = ctx.enter_context(tc.tile_pool(name='persist', bufs=1))
            ps = ctx.enter_context(tc.tile_pool(name='ps', bufs=8, space='PSUM'))

            def big_tile():
                return big.tile([CHUNK, 4096], F32, tag='big', name='bigt')

            def sm_tile(w=1):
                return sm.tile([CHUNK, 16], F32, tag='sm', name='smt')[:, 0:w]

            def ps_tile():
                return ps.tile([128, 512], F32, tag='ps', name='pst')

            # ---------- persistent small loads ----------
            ident = persist.tile([128, 128], F32, tag='ident')
            make_identity(nc, ident[:])
            xT = persist.tile([128, 8 * CHUNK], F32, tag='xT')
            for dt in range(8):
                nc.sync.dma_start(xT[:, dt*CHUNK:(dt+1)*CHUNK],
                                  xT_c[dt*128:(dt+1)*128, :])
            xn = persist.tile([CHUNK, D], F32, tag='xn')
            nc.sync.dma_start(xn[:], x_c[:])
            cb = persist.tile([CHUNK, S], F32, tag='cb')
            nc.sync.dma_start(cb[:], cbias[:])
            taus = {}
            for nm, t_ in (('q', ntq), ('k', ntk), ('v', ntv), ('kn', ntkn)):
                tt = persist.tile([CHUNK, 1], F32, tag='tau' + nm)
                nc.sync.dma_start(tt[:], t_[:])
                taus[nm] = tt
            pos_t = {}
            for nm, t_ in (('qk', posT_qk), ('v', posT_v), ('kn', posT_know)):
                tt = persist.tile([P + 1, CHUNK], F32, tag='pos' + nm)
                nc.sync.dma_start(tt[:], t_[:])
                pos_t[nm] = tt
            loss_sb = persist.tile([CHUNK, 8], F32, tag='loss')
            nc.vector.memset(loss_sb[:], 0.0)

            # ---------- helpers ----------
            def dve_ts(out, in0, s1, s2, op0, op1=None, accum=None):
                kw = {}
                if op1 is not None:
                    kw['op1'] = op1
                if accum is not None:
                    kw['accum_out'] = accum
                nc.vector.tensor_scalar(out, in0, s1, s2, op0=op0, **kw)

            def pool_ts(out, in0, s1, s2, op0, op1=None):
                kw = {'op1': op1} if op1 is not None else {}
                nc.gpsimd.tensor_scalar(out, in0, s1, s2, op0=op0, **kw)

            def pool_tt(out, a, b, op):
                nc.gpsimd.tensor_tensor(out, a, b, op=op)

            def evict(dst, src):  # PSUM -> SBUF on ACT (Identity works, Copy broken)
                nc.scalar.activation(dst, src, Act.Identity, bias=0.0, scale=1.0)

            def count_le_dve(halves, mid, hw):
                tot = None
                for h_ in halves:
                    cnt = sm_tile()
                    junk = big_tile()
                    dve_ts(junk[:, :hw], h_, mid[:, 0:1], 0.0, Alu.is_le,
                           Alu.add, cnt[:, 0:1])
                    if tot is None:
                        tot = cnt
                    else:
                        c2 = sm_tile()
                        pool_tt(c2[:, 0:1], tot[:, 0:1], cnt[:, 0:1], Alu.add)
                        tot = c2
                return tot

            def count_sgn_act(halves, mid, hw):
                """sum(sign(a - mid)) over the row (all halves)."""
                nmid = sm_tile()
                pool_ts(nmid[:, 0:1], mid[:, 0:1], -1.0, None, Alu.mult)
                tot = None
                for h_ in halves:
                    sg = sm_tile()
                    junk = big_tile()
                    nc.scalar.activation(junk[:, :hw], h_, Act.Sign,
                                         bias=nmid[:, 0:1], scale=1.0,
                                         accum_out=sg[:, 0:1])
                    if tot is None:
                        tot = sg
                    else:
                        t2 = sm_tile()
                        pool_tt(t2[:, 0:1], tot[:, 0:1], sg[:, 0:1], Alu.add)
                        tot = t2
                return tot

            def bisect_generic(Ainit, Binit, iters, make_pred):
                """pred true -> A = mid ; else B = mid. returns final A."""
                A, B = Ainit, Binit
                for _ in range(iters):
                    mid0 = sm_tile()
                    pool_tt(mid0[:, 0:1], A[:, 0:1], B[:, 0:1], Alu.add)
                    mid = sm_tile()
                    pool_ts(mid[:, 0:1], mid0[:, 0:1], 0.5, None, Alu.mult)
                    pred = make_pred(mid)
                    predn = sm_tile()
                    pool_ts(predn[:, 0:1], pred[:, 0:1], -1.0, 1.0, Alu.mult, Alu.add)
                    dA = sm_tile(); dB = sm_tile()
                    pool_tt(dA[:, 0:1], mid[:, 0:1], A[:, 0:1], Alu.subtract)
                    pool_tt(dB[:, 0:1], mid[:, 0:1], B[:, 0:1], Alu.subtract)
                    uA = sm_tile(); uB = sm_tile()
                    pool_tt(uA[:, 0:1], pred[:, 0:1], dA[:, 0:1], Alu.mult)
                    pool_tt(uB[:, 0:1], predn[:, 0:1], dB[:, 0:1], Alu.mult)
                    A2 = sm_tile(); B2 = sm_tile()
                    pool_tt(A2[:, 0:1], A[:, 0:1], uA[:, 0:1], Alu.add)
                    pool_tt(B2[:, 0:1], B[:, 0:1], uB[:, 0:1], Alu.add)
                    A, B = A2, B2
                return A

            def seg_reduce(halves, hw, nseg_per_half, op):
                tot = len(halves) * nseg_per_half
                segs = sm.tile([CHUNK, 128], F32, tag='segs', bufs=2, name='segs')
                for i, h_ in enumerate(halves):
                    hv = h_.rearrange("p (s i) -> p s i", s=nseg_per_half)
                    nc.vector.tensor_reduce(
                        segs[:, i*nseg_per_half:(i+1)*nseg_per_half], hv, op=op,
                        axis=mybir.AxisListType.X)
                return segs, tot

            def row_reduce(t_, op, w):
                r = sm_tile()
                nc.vector.tensor_reduce(r[:, 0:1], t_[:, :w], op=op,
                                        axis=mybir.AxisListType.X)
                return r

            # ================= one neuron-table stage =================
            def table_stage(nn, nhalves, npos_d, pos_tile, tabT_d, tab_d,
                            lhsT_tile, k_cand, t_dist_iters, gates,
                            dist_on_act=False):
                hw = nn // nhalves
                nt_h = hw // 512
                # --- distances ---
                dS = []
                for hf in range(nhalves):
                    dst = big_tile()
                    for ntile in range(nt_h):
                        pd = ps_tile()
                        col0 = hf * hw + ntile * 512
                        np_ = st_t.tile([P + 1, 512], F32, tag='st_t', name='np_tile')
                        nc.sync.dma_start(np_[:], npos_d[:, col0:col0+512])
                        nc.tensor.matmul(pd[:], pos_tile[:], np_[:],
                                         start=True, stop=True)
                        evict(dst[:, ntile*512:(ntile+1)*512], pd[:])
                    dS.append(dst)
                dhalves = [t_[:, :hw] for t_ in dS]
                # --- candidate threshold (bisect; bracket from segment minima) ---
                segs, totseg = seg_reduce(dhalves, hw, hw // 64, Alu.min)
                lo0 = row_reduce(segs, Alu.min, totseg)
                hi0 = row_reduce(segs, Alu.max, totseg)

                def dist_pred(mid):
                    pred = sm_tile()
                    if dist_on_act:
                        sg = count_sgn_act(dhalves, mid, hw)
                        pool_ts(pred[:, 0:1], sg[:, 0:1],
                                float(nn - 2*k_cand + 1), None, Alu.is_le)
                    else:
                        cnt = count_le_dve(dhalves, mid, hw)
                        pool_ts(pred[:, 0:1], cnt[:, 0:1],
                                float(k_cand) - 0.5, None, Alu.is_ge)
                    return pred

                t64 = bisect_generic(hi0, lo0, t_dist_iters, dist_pred)

                # --- act matmuls + masked eviction ---
                am = []
                for hf in range(nhalves):
                    mn = big_tile()
                    dve_ts(mn[:, :hw], dS[hf][:, :hw], t64[:, 0:1], NEG,
                           Alu.is_gt, Alu.mult)
                    dst = big_tile()
                    pa = [ps_tile() for _ in range(nt_h)]
                    for dt in range(8):
                        for ntile in range(nt_h):
                            col0 = hf * hw + ntile * 512
                            tt_ = st_t.tile([128, 512], F32, tag='st_t', name='stt_tile')
                            nc.sync.dma_start(tt_[:],
                                              tabT_d[dt*128:(dt+1)*128, col0:col0+512])
                            nc.tensor.matmul(
                                pa[ntile][:],
                                lhsT_tile[:, dt*CHUNK:(dt+1)*CHUNK], tt_[:],
                                start=(dt == 0), stop=(dt == 7))
                    for ntile in range(nt_h):
                        nc.vector.scalar_tensor_tensor(
                            dst[:, ntile*512:(ntile+1)*512], in0=pa[ntile][:],
                            scalar=0.0, in1=mn[:, ntile*512:(ntile+1)*512],
                            op0=Alu.add, op1=Alu.add)
                    am.append(dst)
                ahalves = [t_[:, :hw] for t_ in am]

                # --- gate threshold a_thr + row max act (shared by all gates:
                #     the keep-set is the top-k_keep by act, tau-independent) ---
                k_keep = gates[0][2]
                if nhalves == 1 and k_keep <= 32:
                    rounds = k_keep // 8
                    m8s = []
                    prev = am[0][:, :hw]
                    for r in range(rounds):
                        m8 = sm.tile([CHUNK, 8], F32, tag='m8', bufs=8, name='m8')
                        nc.vector.max(out=m8[:], in_=prev)
                        m8s.append(m8)
                        if r < rounds - 1:
                            wk = big_tile()
                            nc.vector.match_replace(
                                out=wk[:, :hw], in_to_replace=m8[:],
                                in_values=prev, imm_value=NEG)
                            prev = wk[:, :hw]
                    a_thr = m8s[-1][:, 7:8]
                    a_max = m8s[0][:, 0:1]
                else:
                    sga, totsg = seg_reduce(ahalves, hw, hw // 128, Alu.max)
                    alo = row_reduce(sga, Alu.min, totsg)
                    ahi = row_reduce(sga, Alu.max, totsg)
                    a_max = ahi

                    def act_pred(mid):
                        sg = count_sgn_act(ahalves, mid, hw)
                        pred = sm_tile()
                        pool_ts(pred[:, 0:1], sg[:, 0:1],
                                float(2*k_keep - 1 - nn), None, Alu.is_ge)
                        return pred
                    a_thr = bisect_generic(alo, ahi, T_AK, act_pred)

                wTs = {}
                for gname, negtau, k_keep, want_loss, loss_base in gates:
                    # --- alpha = tanh(max_eg) / (sum(eg_kept) + 1e-8) ---
                    t1 = sm_tile()
                    nc.scalar.activation(t1[:, 0:1], a_max[:, 0:1], Act.Exp,
                                         bias=negtau[:, 0:1], scale=1.0)
                    meg = sm_tile()
                    dve_ts(meg[:, 0:1], t1[:, 0:1], 1.0, 0.0, Alu.subtract, Alu.max)
                    stren = sm_tile()
                    nc.scalar.activation(stren[:, 0:1], meg[:, 0:1], Act.Tanh,
                                         bias=0.0, scale=1.0)
                    Ssum = None
                    egks = []
                    for hf in range(nhalves):
                        e = big_tile()
                        nc.scalar.activation(e[:, :hw], am[hf][:, :hw], Act.Exp,
                                             bias=negtau[:, 0:1], scale=1.0)
                        eg = big_tile()
                        dve_ts(eg[:, :hw], e[:, :hw], 1.0, 0.0, Alu.subtract, Alu.max)
                        egk = big_tile()
                        nc.vector.scalar_tensor_tensor(
                            egk[:, :hw], in0=am[hf][:, :hw], scalar=a_thr[:, 0:1],
                            in1=eg[:, :hw], op0=Alu.is_ge, op1=Alu.mult)
                        s_ = sm_tile()
                        nc.vector.tensor_reduce(s_[:, 0:1], egk[:, :hw], op=Alu.add,
                                                axis=mybir.AxisListType.X)
                        if Ssum is None:
                            Ssum = s_
                        else:
                            s2 = sm_tile()
                            pool_tt(s2[:, 0:1], Ssum[:, 0:1], s_[:, 0:1], Alu.add)
                            Ssum = s2
                        egks.append(egk)
                    sp = sm_tile()
                    pool_ts(sp[:, 0:1], Ssum[:, 0:1], 1e-8, None, Alu.add)
                    rec = sm_tile()
                    nc.vector.reciprocal(rec[:, 0:1], sp[:, 0:1])
                    alpha = sm_tile()
                    pool_tt(alpha[:, 0:1], stren[:, 0:1], rec[:, 0:1], Alu.mult)

                    if want_loss:
                        ldt = None
                        for hf in range(nhalves):
                            pj = big_tile()
                            nc.vector.tensor_tensor(pj[:, :hw], egks[hf][:, :hw],
                                                    dS[hf][:, :hw], op=Alu.mult)
                            l_ = sm_tile()
                            nc.vector.tensor_reduce(l_[:, 0:1], pj[:, :hw],
                                                    op=Alu.add,
                                                    axis=mybir.AxisListType.X)
                            if ldt is None:
                                ldt = l_
                            else:
                                l2 = sm_tile()
                                pool_tt(l2[:, 0:1], ldt[:, 0:1], l_[:, 0:1], Alu.add)
                                ldt = l2
                        pool_tt(loss_sb[:, loss_base:loss_base+1],
                                ldt[:, 0:1], alpha[:, 0:1], Alu.mult)
                        pool_tt(loss_sb[:, loss_base+1:loss_base+2],
                                alpha[:, 0:1], Ssum[:, 0:1], Alu.mult)

                    # --- w = act*alpha*eg_kept, transposed per half ---
                    wT_halves = []
                    for hf in range(nhalves):
                        w_ = big_tile()
                        nc.vector.scalar_tensor_tensor(
                            w_[:, :hw], in0=am[hf][:, :hw], scalar=alpha[:, 0:1],
                            in1=egks[hf][:, :hw], op0=Alu.mult, op1=Alu.mult)
                        wT = big_tile()
                        nblk = hw // 128
                        for b0 in range(0, nblk, 4):
                            pt = ps_tile()
                            for b in range(b0, b0 + 4):
                                nc.tensor.transpose(pt[:, (b-b0)*128:(b-b0+1)*128],
                                                    w_[:, b*128:(b+1)*128], ident[:])
                            evict(wT[:, b0*128:(b0+4)*128], pt[:])
                        wT_halves.append(wT)
                    wTs[gname] = wT_halves

                # --- recon: stream native table once; matmul all gates ---
                outs = {g: [ps_tile(), ps_tile()] for g in wTs}
                ntt = nn // 128
                blocks_per_half = hw // 128
                for ntile in range(ntt):
                    tnat = st_n.tile([128, D], F32, tag='st_n', name='stn_tile')
                    nc.sync.dma_start(tnat[:], tab_d[ntile*128:(ntile+1)*128, :])
                    hf = ntile // blocks_per_half
                    lb = ntile % blocks_per_half
                    for g in wTs:
                        for half in range(2):
                            nc.tensor.matmul(
                                outs[g][half][:],
                                wTs[g][hf][:, lb*128:(lb+1)*128],
                                tnat[:, half*512:(half+1)*512],
                                start=(ntile == 0), stop=(ntile == ntt - 1))
                res = {}
                for g in wTs:
                    dst = big_tile()
                    evict(dst[:, 0:512], outs[g][0][:])
                    evict(dst[:, 512:1024], outs[g][1][:])
                    res[g] = dst
                return res

            # ---------------- qk stage ----------------
            qk_out = table_stage(
                NQK, 1, nposT_qk, pos_t['qk'], tabT_qk, tab_qk,
                xT, NCAND_QK, T_D,
                gates=[('Q', taus['q'], MAXK_QK, True, 0),
                       ('K', taus['k'], MAXK_QK, False, 0)])

            # ---------------- v stage (distance counts on ACT) ----------------
            v_out = table_stage(
                NV, 1, nposT_v, pos_t['v'], tabT_v, tab_v,
                xT, NCAND_V, T_D,
                gates=[('V', taus['v'], MAXK_V, True, 2)],
                dist_on_act=True)

            # ---------------- transpose Q,K ; allgather K,V ----------------
            def transpose_1024(src, dst):
                for b0 in range(0, 8, 4):
                    pt = ps_tile()
                    for b in range(b0, b0+4):
                        nc.tensor.transpose(pt[:, (b-b0)*128:(b-b0+1)*128],
                                            src[:, b*128:(b+1)*128], ident[:])
                    evict(dst[:, b0*128:(b0+4)*128], pt[:])

            QT = persist.tile([128, 8 * CHUNK], F32, tag='QT')
            transpose_1024(qk_out['Q'], QT)
            KT = big_tile()
            transpose_1024(qk_out['K'], KT)
            for dt in range(8):
                nc.sync.dma_start(kt_bounce[dt*128:(dt+1)*128, :],
                                  KT[:, dt*128:(dt+1)*128])
            nc.sync.dma_start(v_bounce[:], v_out['V'][:, 0:1024])
            nc.gpsimd.collective_compute(
                'AllGather', mybir.AluOpType.bypass, replica_groups=groups,
                ins=[kt_bounce[:]], outs=[kt_all[:]])
            nc.gpsimd.collective_compute(
                'AllGather', mybir.AluOpType.bypass, replica_groups=groups,
                ins=[v_bounce[:]], outs=[v_all[:]])

            # ---------------- attention ----------------
            oT = persist.tile([128, 8 * CHUNK], F32, tag='oT')
            for g in range(4):
                KTg = big_tile()
                Vg = big_tile()
                for j in range(n_cores):
                    for b in range(2):
                        nc.sync.dma_start(
                            KTg[:, (j*2+b)*128:(j*2+b+1)*128],
                            kt_all[j, 128*(2*g+b):128*(2*g+b+1), :])
                    nc.sync.dma_start(Vg[:, j*256:(j+1)*256],
                                      v_all[j, :, 256*g:256*(g+1)])
                for hh in range(4):
                    h = 4*g + hh
                    pb = (h % 2) * 64
                    b = (h % 4) // 2
                    sc0, sc1 = ps_tile(), ps_tile()
                    for j in range(8):
                        dstp = sc0 if j < 4 else sc1
                        nc.tensor.matmul(
                            dstp[:, (j % 4)*128:(j % 4 + 1)*128],
                            QT[pb:pb+64, 128*(h//2):128*(h//2)+128],
                            KTg[pb:pb+64, (j*2+b)*128:(j*2+b+1)*128],
                            start=True, stop=True)
                    ssb = big_tile()
                    nc.vector.scalar_tensor_tensor(
                        ssb[:, 0:512], in0=sc0[:], scalar=0.0,
                        in1=cb[:, 0:512], op0=Alu.add, op1=Alu.add)
                    nc.vector.scalar_tensor_tensor(
                        ssb[:, 512:1024], in0=sc1[:], scalar=0.0,
                        in1=cb[:, 512:1024], op0=Alu.add, op1=Alu.add)
                    rm = row_reduce(ssb, Alu.max, 1024)
                    nm = sm_tile()
                    pool_ts(nm[:, 0:1], rm[:, 0:1], -0.125, None, Alu.mult)
                    e = big_tile()
                    ssum = sm_tile()
                    nc.scalar.activation(e[:, 0:1024], ssb[:, 0:1024], Act.Exp,
                                         bias=nm[:, 0:1], scale=0.125,
                                         accum_out=ssum[:, 0:1])
                    winv = sm_tile()
                    nc.vector.reciprocal(winv[:, 0:1], ssum[:, 0:1])
                    wn = big_tile()
                    dve_ts(wn[:, 0:1024], e[:, 0:1024], winv[:, 0:1], None, Alu.mult)
                    wT_ = big_tile()
                    for b0 in range(0, 8, 4):
                        pt = ps_tile()
                        for j in range(b0, b0+4):
                            nc.tensor.transpose(pt[:, (j-b0)*128:(j-b0+1)*128],
                                                wn[:, j*128:(j+1)*128], ident[:])
                        evict(wT_[:, b0*128:(b0+4)*128], pt[:])
                    po = ps.tile([64, 128], F32, tag='ps', name='po')
                    for j in range(8):
                        nc.tensor.matmul(po[:],
                                         Vg[:, j*256 + hh*64: j*256 + hh*64 + 64],
                                         wT_[:, j*128:(j+1)*128],
                                         start=(j == 0), stop=(j == 7))
                    evict(oT[pb:pb+64, 128*(h//2):128*(h//2)+128], po[:])

            # attn_out = oT.T @ expO ; h = x + attn_out
            pao = [ps_tile(), ps_tile()]
            for dit in range(8):
                eo = st_n.tile([128, D], F32, tag='st_n', name='stn_tile')
                nc.sync.dma_start(eo[:], expO[dit*128:(dit+1)*128, :])
                for half in range(2):
                    nc.tensor.matmul(pao[half][:], oT[:, dit*128:(dit+1)*128],
                                     eo[:, half*512:(half+1)*512],
                                     start=(dit == 0), stop=(dit == 7))
            h_sb = persist.tile([CHUNK, D], F32, tag='h')
            for half in range(2):
                nc.vector.scalar_tensor_tensor(
                    h_sb[:, half*512:(half+1)*512], in0=pao[half][:], scalar=0.0,
                    in1=xn[:, half*512:(half+1)*512], op0=Alu.add, op1=Alu.add)
            hT = persist.tile([128, 8 * CHUNK], F32, tag='hT')
            transpose_1024(h_sb, hT)

            # ---------------- know stage ----------------
            kn_out = table_stage(
                NK, 2, nposT_know, pos_t['kn'], tabT_know, tab_know,
                hT, NCAND_KNOW, T_DK,
                gates=[('KN', taus['kn'], MAXK_KNOW, True, 4)])

            fin = big_tile()
            nc.vector.tensor_tensor(fin[:, 0:1024], kn_out['KN'][:, 0:1024],
                                    h_sb[:, 0:1024], op=Alu.add)
            nc.sync.dma_start(out_c[:], fin[:, 0:1024])
            nc.sync.dma_start(loss_c[:], loss_sb[:])

    nc.finalize()
    return nc


@functools.lru_cache(maxsize=1)
def _get_nc(n_cores=NCORES):
    return _build_program(n_cores)


def _host_prep(inputs):
    x = np.ascontiguousarray(inputs['x'][0], np.float32)
    xT = np.ascontiguousarray(x.T)

    def aug_posT(pos):
        a = np.empty((P + 1, S), np.float32)
        a[:P] = pos.T
        a[P] = 1.0
        return a

    def aug_nposT(npos):
        n = npos.shape[0]
        a = np.empty((P + 1, n), np.float32)
        a[:P] = -2.0 * npos.T
        a[P] = (npos.astype(np.float32)**2).sum(-1)
        return a

    pq = aug_posT(inputs['qk_pos'][0])
    pv = aug_posT(inputs['v_pos'][0])
    pk = aug_posT(inputs['know_pos'][0])
    gpos = np.arange(S, dtype=np.int64)

    rep = {
        'nposT_qk': aug_nposT(inputs['npos_qk']),
        'nposT_v': aug_nposT(inputs['npos_v']),
        'nposT_know': aug_nposT(inputs['npos_know']),
        'tabT_qk': np.ascontiguousarray(inputs['qk_neurons'].T),
        'tab_qk': np.ascontiguousarray(inputs['qk_neurons']),
        'tabT_v': np.ascontiguousarray(inputs['v_neurons'].T),
        'tab_v': np.ascontiguousarray(inputs['v_neurons']),
        'tabT_know': np.ascontiguousarray(inputs['know_neurons'].T),
        'tab_know': np.ascontiguousarray(inputs['know_neurons']),
        'expO': np.ascontiguousarray(inputs['expand_O']),
    }
    per_core = []
    for c in range(NCORES):
        sl = slice(c*CHUNK, (c+1)*CHUNK)
        cbm = np.where(gpos[None, :] <= gpos[sl][:, None], 0.0, NEG).astype(np.float32)
        m = {
            'x_c': x[sl].copy(),
            'xT_c': np.ascontiguousarray(xT[:, sl]),
            'posT_qk': np.ascontiguousarray(pq[:, sl]),
            'posT_v': np.ascontiguousarray(pv[:, sl]),
            'posT_know': np.ascontiguousarray(pk[:, sl]),
            'negtau_q': np.ascontiguousarray(-inputs['tau_q'][0][sl]),
            'negtau_k': np.ascontiguousarray(-inputs['tau_k'][0][sl]),
            'negtau_v': np.ascontiguousarray(-inputs['tau_v'][0][sl]),
            'negtau_know': np.ascontiguousarray(-inputs['tau_know'][0][sl]),
            'causal_bias': cbm,
        }
        m.update(rep)
        per_core.append(m)
    return per_core


_PER_CORE = {'x_c', 'xT_c', 'posT_qk', 'posT_v', 'posT_know', 'negtau_q',
             'negtau_k', 'negtau_v', 'negtau_know', 'causal_bias'}


@functools.lru_cache(maxsize=1)
def _make_runner():
    """jit-compiled SPMD executable with replicated table inputs.

    Returns (fn, in_names, out_names, zero_outs). fn takes device (or host)
    arrays in in_names order followed by zero output buffers, returns the
    gathered outputs (concat over cores on axis 0).
    """
    import jax
    from jax.sharding import Mesh, PartitionSpec
    from jax.experimental.shard_map import shard_map
    from concourse import bass2jax
    import concourse.mybir as mybir

    nc = _get_nc()
    bass2jax.install_neuronx_cc_hook()

    pname = nc.partition_id_tensor.name if nc.partition_id_tensor else None
    in_names, out_names, out_avals, zero_outs = [], [], [], []
    for alloc in nc.m.functions[0].allocations:
        if not isinstance(alloc, mybir.MemoryLocationSet):
            continue
        name = alloc.memorylocations[0].name
        if alloc.kind == 'ExternalInput':
            if name != pname:
                in_names.append(name)
        elif alloc.kind == 'ExternalOutput':
            out_names.append(name)
            shape = tuple(alloc.tensor_shape)
            dt = mybir.dt.np(alloc.dtype)
            out_avals.append(jax.core.ShapedArray(shape, dt))
            zero_outs.append(np.zeros(shape, dt))
    n_params = len(in_names)
    all_in = in_names + out_names + ([pname] if pname else [])

    def _body(*args):
        operands = list(args)
        if pname:
            operands.append(bass2jax.partition_id_tensor())
        outs = bass2jax._bass_exec_p.bind(
            *operands, out_avals=tuple(out_avals), in_names=tuple(all_in),
            out_names=tuple(out_names), lowering_input_output_aliases=(),
            sim_require_finite=True, sim_require_nnan=True, nc=nc)
        return tuple(outs)

    mesh = Mesh(np.asarray(jax.devices()[:NCORES]), ('core',))
    in_specs = tuple(
        PartitionSpec('core') if nm in _PER_CORE else PartitionSpec()
        for nm in in_names) + (PartitionSpec('core'),) * len(out_names)
    out_specs = (PartitionSpec('core'),) * len(out_names)
    fn = jax.jit(
        shard_map(_body, mesh=mesh, in_specs=in_specs, out_specs=out_specs,
                  check_rep=False),
        donate_argnums=tuple(range(n_params, n_params + len(out_names))))
    return fn, in_names, out_names, zero_outs, mesh, in_specs


def device_args(inputs):
    """Host arrays in runner order: per-core inputs concat over cores."""
    per_core = _host_prep(inputs)
    _, in_names, _, zero_outs, _, _ = _make_runner()
    args = []
    for nm in in_names:
        if nm in _PER_CORE:
            args.append(np.concatenate([per_core[c][nm] for c in range(NCORES)],
                                       axis=0))
        else:
            args.append(per_core[0][nm])
    return args, [z.copy() for z in zero_outs]


def run_device(inputs):
    import jax
    from jax.sharding import NamedSharding
    fn, in_names, out_names, zero_outs, mesh, in_specs = _make_runner()
    args, zo = device_args(inputs)
    # pre-place with the exact shardings so no per-call re-layout happens
    args = [jax.device_put(a, NamedSharding(mesh, sp))
            for a, sp in zip(args, in_specs)]
    res = fn(*args, *zo)
    res = {nm: np.asarray(r) for nm, r in zip(out_names, res)}
    outs = [res['out_c'][c*CHUNK:(c+1)*CHUNK] for c in range(NCORES)]
    losses = [res['loss_c'][c*CHUNK:(c+1)*CHUNK] for c in range(NCORES)]
    return outs, losses


def assemble(inputs, outs, losses):
    out = np.concatenate(outs, axis=0)[None]
    lp = np.concatenate(losses, axis=0)
    p2_qk = (inputs['qk_pos'][0].astype(np.float32)**2).sum(-1)
    p2_v = (inputs['v_pos'][0].astype(np.float32)**2).sum(-1)
    p2_k = (inputs['know_pos'][0].astype(np.float32)**2).sum(-1)
    attn_loss = (lp[:, 0] + p2_qk*lp[:, 1] + lp[:, 2] + p2_v*lp[:, 3]).sum() / (8*128*64)
    know_loss = (lp[:, 4] + p2_k*lp[:, 5]).sum() / (8*128*128)
    return out.astype(np.float32), np.float32(attn_loss + know_loss)


def kernel(**inputs):
    inputs = {k: np.asarray(v, np.float32) for k, v in inputs.items()}
    outs, losses = run_device(inputs)
    return assemble(inputs, outs, losses)
